# revision 7
# baseline (speedup 1.0000x reference)
"""Cross-attention Trainium2 Bass kernel.

Problem: nn_CrossAttention (B=4, T=S=2048, C=512, H=16, D=32), fp32.

Sharding: 8 cores, SPMD. Core i handles batch b = i//2 and query-row half
th = i%2 (1024 query rows), with the full context for that batch.  K/V
projections are recomputed per core pair (cheaper than communicating).
No collectives; each core writes a disjoint slice of the output.

Per-core dataflow (all fp32):
  A) Projections via PE:  Q^T[hd,t], K^T[hd,s] (lhsT = W chunks,
     rhs = x^T / ctx^T chunks), V[s,hd] (lhsT = ctx^T chunk, rhs = Wv).
     Biases folded in via an appended ones-row of x^T/ctx^T and a bias
     row in the weight matrices (contraction padded 512->640).
  B) Attention in scores^T layout [s, t] (softmax axis = partitions):
     - QK: lhsT = K^T_h chunk [d=32, s=128], rhs = Q^T_h [32, tb] --
       4 heads row-tiled (tile_position=(32j,0)).
     - exp via ScalarE (scale=1/sqrt(D) folded in, no max subtraction:
       scores are ~N(0,1) so fp32 exp is safe).
     - PV: lhsT = V chunk [s=128, 32], rhs = expS^T [s=128, tb] -- 4 heads
       col-tiled (tile_position=(0,32j)) accumulating AO^T in PSUM.
     - denominators: ones-column matmuls (M=1) col-tiled, accumulated in a
       PSUM bank whose other rows are memset to 1.0 (NaN-safe reciprocal).
     - reciprocal on VectorE, broadcast 4 rows -> 32-row blocks via an
       indicator matmul on PE, normalize AO^T with a VectorE multiply.
  C) Out-proj: lhsT = AO^T_norm chunks, rhs = Wo chunks (+ bias row via
     ones-row matmul), result is already in natural [t, 512] layout -> DMA.
"""

import numpy as np

import concourse.bass as bass
import concourse.mybir as mybir
import concourse.tile as tile
from concourse import bacc

F32 = mybir.dt.float32

B, T, S_FULL, C, H, D = 4, 2048, 2048, 512, 16, 32
TC = 1024          # query rows per core
CA = 640           # padded contraction dim (512 + bias row + pad)
NCH = CA // 128    # 5 contraction chunks
NHG = 4            # head groups (4 heads each)
SCALE = float(D) ** -0.5


def build_nc(tc_rows=TC, s_len=S_FULL, debug=False):
    """Build the single-core Bass program (same program runs on all 8 cores)."""
    tb_size = min(512, tc_rows)      # t block for attention
    ntb = tc_rows // tb_size
    nsc = s_len // 128               # s chunks
    ntt = tc_rows // 128             # t tiles for out-proj

    nc = bacc.Bacc("TRN2", target_bir_lowering=False, debug=debug)

    xT = nc.dram_tensor("xT", [CA, tc_rows], F32, kind="ExternalInput").ap()
    ctxT = nc.dram_tensor("ctxT", [CA, s_len], F32, kind="ExternalInput").ap()
    wq = nc.dram_tensor("wq", [CA, C], F32, kind="ExternalInput").ap()
    wk = nc.dram_tensor("wk", [CA, C], F32, kind="ExternalInput").ap()
    wv = nc.dram_tensor("wv", [CA, C], F32, kind="ExternalInput").ap()
    wo = nc.dram_tensor("wo", [CA, C], F32, kind="ExternalInput").ap()
    ind = nc.dram_tensor("ind", [128, 128], F32, kind="ExternalInput").ap()
    ones = nc.dram_tensor("ones", [128, 128], F32, kind="ExternalInput").ap()
    out = nc.dram_tensor("out", [tc_rows, C], F32, kind="ExternalOutput").ap()

    Exp = mybir.ActivationFunctionType.Exp

    with tile.TileContext(nc) as tc:
        # ---------------- persistent tiles ----------------
        with tc.tile_pool(name="pers", bufs=1) as pers:
            qT_t = pers.tile([128, NHG, tc_rows], F32)   # Q^T by hd-chunk
            kT_t = pers.tile([128, NHG, s_len], F32)     # K^T by hd-chunk
            v_t = pers.tile([128, nsc, C], F32)          # V by s-chunk
            aon_t = pers.tile([128, NHG, tc_rows], F32)  # normalized AO^T
            wo_t = pers.tile([128, NCH, C], F32)
            ind_t = pers.tile([128, 128], F32)
            ones_t = pers.tile([128, 128], F32)

            nc.sync.dma_start(out=wo_t[:], in_=wo.rearrange("(c p) n -> p c n", p=128))
            nc.sync.dma_start(out=ind_t[:], in_=ind)
            nc.sync.dma_start(out=ones_t[:], in_=ones)

            # ---------------- phase A: projections ----------------
            with (
                tc.tile_pool(name="phA", bufs=1) as pa,
                tc.tile_pool(name="psA", bufs=4, space="PSUM") as psA,
            ):
                xT_t = pa.tile([128, NCH, tc_rows], F32)
                ctxT_t = pa.tile([128, NCH, s_len], F32)
                wq_t = pa.tile([128, NCH, C], F32)
                wk_t = pa.tile([128, NCH, C], F32)
                wv_t = pa.tile([128, NCH, C], F32)
                nc.sync.dma_start(out=xT_t[:], in_=xT.rearrange("(c p) n -> p c n", p=128))
                nc.sync.dma_start(out=ctxT_t[:], in_=ctxT.rearrange("(c p) n -> p c n", p=128))
                nc.sync.dma_start(out=wq_t[:], in_=wq.rearrange("(c p) n -> p c n", p=128))
                nc.sync.dma_start(out=wk_t[:], in_=wk.rearrange("(c p) n -> p c n", p=128))
                nc.sync.dma_start(out=wv_t[:], in_=wv.rearrange("(c p) n -> p c n", p=128))

                # Q^T and K^T: out[hd-chunk m, block] = sum_c W[:,c,m].T @ inT[:,c,blk]
                for m in range(NHG):
                    for b2 in range(tc_rows // tb_size):
                        ps = psA.tile([128, tb_size], F32, tag="prj")
                        for c in range(NCH):
                            nc.tensor.matmul(
                                ps[:],
                                wq_t[:, c, 128 * m : 128 * (m + 1)],
                                xT_t[:, c, tb_size * b2 : tb_size * (b2 + 1)],
                                start=(c == 0),
                                stop=(c == NCH - 1),
                            )
                        nc.any.tensor_copy(
                            qT_t[:, m, tb_size * b2 : tb_size * (b2 + 1)], ps[:]
                        )
                    kb = min(512, s_len)
                    for b2 in range(s_len // kb):
                        ps = psA.tile([128, kb], F32, tag="prj")
                        for c in range(NCH):
                            nc.tensor.matmul(
                                ps[:],
                                wk_t[:, c, 128 * m : 128 * (m + 1)],
                                ctxT_t[:, c, kb * b2 : kb * (b2 + 1)],
                                start=(c == 0),
                                stop=(c == NCH - 1),
                            )
                        nc.any.tensor_copy(kT_t[:, m, kb * b2 : kb * (b2 + 1)], ps[:])

                # V: out[s-chunk sb, :] = ctx chunk rows x Wv
                for sb in range(nsc):
                    ps = psA.tile([128, C], F32, tag="prj")
                    for c in range(NCH):
                        nc.tensor.matmul(
                            ps[:],
                            ctxT_t[:, c, 128 * sb : 128 * (sb + 1)],
                            wv_t[:, c, :],
                            start=(c == 0),
                            stop=(c == NCH - 1),
                        )
                    nc.any.tensor_copy(v_t[:, sb, :], ps[:])

            # ---------------- phase B: attention ----------------
            with (
                tc.tile_pool(name="phB", bufs=1) as pb,
                tc.tile_pool(name="st_ps", bufs=3, space="PSUM") as st_pool,
                tc.tile_pool(name="acc_ps", bufs=1, space="PSUM") as acc_pool,
            ):
                for g in range(NHG):
                    for tb in range(ntb):
                        tsl = slice(tb_size * tb, tb_size * (tb + 1))
                        ao = acc_pool.tile([128, tb_size], F32, tag="ao")
                        sm = acc_pool.tile([128, tb_size], F32, tag="sm")
                        # rows not written by the M=1 sums matmuls -> 1.0 so
                        # reciprocal stays finite (bcast matmul multiplies by 0;
                        # matmul start=True only resets elements it writes)
                        nc.vector.memset(sm[:], 1.0)

                        for cix in range(nsc):
                            ssl = slice(128 * cix, 128 * (cix + 1))
                            for pair in range(2):
                                st = st_pool.tile([128, 2, tb_size], F32, tag="st")
                                ex = pb.tile([128, 2, tb_size], F32, tag="ex", bufs=3)
                                for j2 in range(2):
                                    j = 2 * pair + j2
                                    nc.tensor.matmul(
                                        st[:, j2, :],
                                        kT_t[32 * j : 32 * (j + 1), g, ssl],
                                        qT_t[32 * j : 32 * (j + 1), g, tsl],
                                        start=True,
                                        stop=True,
                                        tile_position=(32 * j, 0),
                                    )
                                nc.scalar.activation(ex[:], st[:], Exp, scale=SCALE)
                                for j2 in range(2):
                                    j = 2 * pair + j2
                                    h = 4 * g + j
                                    nc.tensor.matmul(
                                        ao[32 * j : 32 * (j + 1), :],
                                        v_t[:, cix, 32 * h : 32 * (h + 1)],
                                        ex[:, j2, :],
                                        start=(cix == 0),
                                        stop=(cix == nsc - 1),
                                        tile_position=(0, 32 * j),
                                        skip_group_check=True,
                                    )
                                    nc.tensor.matmul(
                                        sm[32 * j : 32 * j + 1, :],
                                        ones_t[:, 0:1],
                                        ex[:, j2, :],
                                        start=(cix == 0),
                                        stop=(cix == nsc - 1),
                                        tile_position=(0, 32 * j),
                                        skip_group_check=True,
                                    )

                        rc = pb.tile([128, tb_size], F32, tag="rc", bufs=2)
                        nc.vector.reciprocal(rc[:], sm[:])
                        bc = st_pool.tile([128, tb_size], F32, tag="st")
                        nc.tensor.matmul(bc[:], ind_t[:], rc[:], start=True, stop=True)
                        bcs = pb.tile([128, tb_size], F32, tag="bcs", bufs=2)
                        nc.vector.tensor_copy(bcs[:], bc[:])
                        nc.vector.tensor_mul(aon_t[:, g, tsl], ao[:], bcs[:])

            # ---------------- phase C: out-projection ----------------
            with (
                tc.tile_pool(name="phC", bufs=1) as pcp,
                tc.tile_pool(name="psC", bufs=2, space="PSUM") as psC,
            ):
                out_r = out.rearrange("(tt p) n -> tt p n", p=128)
                for tt in range(ntt):
                    po = psC.tile([128, C], F32, tag="po")
                    for m in range(NHG):
                        nc.tensor.matmul(
                            po[:],
                            aon_t[:, m, 128 * tt : 128 * (tt + 1)],
                            wo_t[:, m, :],
                            start=(m == 0),
                            stop=False,
                        )
                    # bias: ones-column (t) x bo row
                    nc.tensor.matmul(
                        po[:],
                        ones_t[0:1, 0:128],
                        wo_t[0:1, NCH - 1, :],
                        start=False,
                        stop=True,
                    )
                    ob = pcp.tile([128, C], F32, tag="ob", bufs=3)
                    nc.any.tensor_copy(ob[:], po[:])
                    nc.sync.dma_start(out=out_r[tt], in_=ob[:])

    nc.compile()
    return nc


# ---------------------------------------------------------------------------
# host side
# ---------------------------------------------------------------------------

def _augment_w(Wmat, bvec):
    Wa = np.zeros((CA, C), np.float32)
    Wa[:C] = np.asarray(Wmat, np.float32)
    Wa[C] = np.asarray(bvec, np.float32)
    return Wa


def _indicator():
    indm = np.zeros((128, 128), np.float32)
    for m in range(128):
        indm[32 * (m // 32), m] = 1.0
    return indm


def _make_in_maps(x, context, Wq, bq, Wk, bk, Wv, bv, Wo, bo, tc_rows=TC, s_len=S_FULL):
    wqa, wka = _augment_w(Wq, bq), _augment_w(Wk, bk)
    wva, woa = _augment_w(Wv, bv), _augment_w(Wo, bo)
    indm = _indicator()
    onesm = np.ones((128, 128), np.float32)
    n_halves = x.shape[1] // tc_rows
    in_maps = []
    for core in range(8):
        b, th = core // n_halves, core % n_halves
        xs = np.asarray(x[b, th * tc_rows : (th + 1) * tc_rows], np.float32)
        xTm = np.zeros((CA, tc_rows), np.float32)
        xTm[:C] = xs.T
        xTm[C] = 1.0
        cs = np.asarray(context[b, :s_len], np.float32)
        cTm = np.zeros((CA, s_len), np.float32)
        cTm[:C] = cs.T
        cTm[C] = 1.0
        in_maps.append(
            dict(xT=xTm, ctxT=cTm, wq=wqa, wk=wka, wv=wva, wo=woa, ind=indm, ones=onesm)
        )
    return in_maps


_NC_CACHE = {}


def _get_nc():
    if "nc" not in _NC_CACHE:
        _NC_CACHE["nc"] = build_nc()
    return _NC_CACHE["nc"]


def _ensure_ntff_hook():
    """The agent image's antenv lacks axon_hooks; synthesize it so
    run_bass_kernel_spmd(trace=True) can capture NTFF profiles."""
    import sys
    import types

    try:
        from antenv.axon_hooks import get_axon_ntff_profile_hook  # noqa: F401
        return
    except ImportError:
        pass
    import antenv
    from trn_agent_boot.trn_boot import _ntff_profile_via_ctypes

    mod = types.ModuleType("antenv.axon_hooks")
    state = {"h": _ntff_profile_via_ctypes("/opt/axon/libaxon_pjrt.so")}
    mod.get_axon_ntff_profile_hook = lambda: state["h"]
    mod.set_axon_ntff_profile_hook = lambda h: state.__setitem__("h", h)
    sys.modules["antenv.axon_hooks"] = mod
    antenv.axon_hooks = mod


def _run(inputs, trace=False):
    from concourse.bass_utils import run_bass_kernel_spmd

    if trace:
        try:
            _ensure_ntff_hook()
        except Exception as e:  # fall back to untraced run
            print(f"ntff hook unavailable: {e}")
            trace = False

    in_maps = _make_in_maps(
        inputs["x"], inputs["context"],
        inputs["Wq"], inputs["bq"], inputs["Wk"], inputs["bk"],
        inputs["Wv"], inputs["bv"], inputs["Wo"], inputs["bo"],
    )
    nc = _get_nc()
    res = run_bass_kernel_spmd(nc, in_maps, list(range(8)), trace=trace)
    x = inputs["x"]
    out = np.empty((B, T, C), np.float32)
    n_halves = T // TC
    for core in range(8):
        b, th = core // n_halves, core % n_halves
        out[b, th * TC : (th + 1) * TC] = res.results[core]["out"]
    return out, res


def kernel(**inputs) -> np.ndarray:
    out, _ = _run(inputs, trace=False)
    return out


def kernel_timed(inputs):
    out, res = _run(inputs, trace=True)
    return out, res.exec_time_ns
